# revision 1
# baseline (speedup 1.0000x reference)
"""Decoder-only transformer (GPT-style, post-LN) forward pass on 8 Trainium2 cores.

Sharding: tokens (batch*seq) are block-sharded 8 ways for the embedding and the
4 transformer layers (core c owns batch c//4, seq chunk c%4 of 512 tokens).
K^T and V are packed into one bf16 buffer and all-gathered per layer within
each batch's 4-core group (one collective per layer, 4 total). The LM head is
token-sharded: each core computes its own 512 tokens against the full vocab,
so no final all-gather is needed. Logits are written bf16 and upcast on host.

Weights (wq/wk/wv/wo/w1/w2/lm_w/tok_emb) are shipped and consumed in bf16;
the residual stream h, layernorms, and all PSUM accumulation stay fp32.
"""

import math
import os

import numpy as np
import ml_dtypes

import concourse.bass as bass
import concourse.bacc as bacc
import concourse.mybir as mybir
import concourse.tile as tile
from concourse.bass_utils import run_bass_kernel_spmd
from concourse.masks import make_identity

# model dims (hardcoded per problem spec)
V, S, D, NL, H = 50257, 2048, 768, 4, 12
HD, DF, B = 64, 3072, 2
NC = 8          # cores
CH = 512        # tokens per core
QT = 4          # 128-token tiles per core
DT = 6          # 128-wide d tiles
FT = 24         # 128-wide dff tiles
VP = 51200      # padded vocab (100 * 512)
NVC = 100       # vocab chunks of 512
RANKS = 4       # cores per batch group
KVROWS = 1280   # 768 (K^T) + 512 (V) rows in the packed per-layer KV buffer
KVW = 780       # H * (HD + 1)

F32 = mybir.dt.float32
F32R = mybir.dt.float32r
BF16 = mybir.dt.bfloat16
I32 = mybir.dt.int32
AX = mybir.AxisListType.X
OP = mybir.AluOpType
AF = mybir.ActivationFunctionType
P = 128

_CACHE = {}


def build():
    nc = bacc.Bacc(None, target_bir_lowering=False, num_devices=NC)

    # ---- kernel I/O ----
    ids = nc.dram_tensor("ids", [P, QT], I32, kind="ExternalInput")
    pe_in = nc.dram_tensor("pe", [P, QT, D], F32, kind="ExternalInput")
    masks_in = nc.dram_tensor("masks", [P, 16, CH], BF16, kind="ExternalInput")
    tok_emb = nc.dram_tensor("tok_emb", [V, D], BF16, kind="ExternalInput")
    wq_d = nc.dram_tensor("wq", [NL, D, D], BF16, kind="ExternalInput")
    wk_d = nc.dram_tensor("wk", [NL, D, D], BF16, kind="ExternalInput")
    wv_d = nc.dram_tensor("wv", [NL, D, D], BF16, kind="ExternalInput")
    wo_d = nc.dram_tensor("wo", [NL, D, D], BF16, kind="ExternalInput")
    w1_d = nc.dram_tensor("w1", [NL, D, DF], BF16, kind="ExternalInput")
    w2_d = nc.dram_tensor("w2", [NL, DF, D], BF16, kind="ExternalInput")
    b1_d = nc.dram_tensor("b1", [NL, DF], F32, kind="ExternalInput")
    b2_d = nc.dram_tensor("b2", [NL, D], BF16, kind="ExternalInput")
    ln1g_d = nc.dram_tensor("ln1_g", [NL, D], F32R, kind="ExternalInput")
    ln1b_d = nc.dram_tensor("ln1_b", [NL, D], F32R, kind="ExternalInput")
    ln2g_d = nc.dram_tensor("ln2_g", [NL, D], F32R, kind="ExternalInput")
    ln2b_d = nc.dram_tensor("ln2_b", [NL, D], F32R, kind="ExternalInput")
    lnfg_d = nc.dram_tensor("lnf_g", [1, D], F32R, kind="ExternalInput")
    lnfb_d = nc.dram_tensor("lnf_b", [1, D], F32R, kind="ExternalInput")
    lmw_d = nc.dram_tensor("lm_w", [D, VP], BF16, kind="ExternalInput")
    lmb_d = nc.dram_tensor("lm_b", [1, VP], BF16, kind="ExternalInput")
    ones_d = nc.dram_tensor("c_ones", [1, P], F32R, kind="ExternalInput")
    logits = nc.dram_tensor("logits", [CH, VP], BF16, kind="ExternalOutput")

    g4 = [[0, 1, 2, 3], [4, 5, 6, 7]]

    with tile.TileContext(nc) as tc:
        with (
            tc.tile_pool(name="pers", bufs=1) as pers,
            tc.tile_pool(name="dram", bufs=1, space="DRAM") as dram,
        ):
            h = pers.tile([P, QT, D], F32, name="h_res")
            ones_sb = pers.tile([1, P], F32R, name="ones_sb")
            nc.sync.dma_start(ones_sb[:], ones_d[:])
            ones_b = pers.tile([1, P], BF16, name="ones_b")
            nc.vector.memset(ones_b[:], 1.0)
            id32 = pers.tile([P, P], F32, name="id32")
            make_identity(nc, id32[:])
            idb = pers.tile([P, P], BF16, name="idb")
            make_identity(nc, idb[:])

            # ---------- embedding: gather + positional encoding ----------
            with tc.tile_pool(name="embp", bufs=1) as ep:
                ids_sb = ep.tile([P, QT], I32)
                nc.sync.dma_start(ids_sb[:], ids[:])
                pe_sb = ep.tile([P, QT, D], F32)
                nc.sync.dma_start(pe_sb[:], pe_in[:])
                for qt in range(QT):
                    emb = ep.tile([P, D], BF16, tag="emb", bufs=2)
                    nc.gpsimd.indirect_dma_start(
                        out=emb[:],
                        out_offset=None,
                        in_=tok_emb[:],
                        in_offset=bass.IndirectOffsetOnAxis(ap=ids_sb[:, qt : qt + 1], axis=0),
                    )
                    nc.vector.tensor_copy(out=h[:, qt, :], in_=emb[:])
                    nc.vector.tensor_tensor(h[:, qt, :], h[:, qt, :], pe_sb[:, qt, :], OP.add)

            # ---------- transformer layers ----------
            from contextlib import ExitStack

            with tc.tile_pool(name="wk", bufs=1) as wk:
                _ps_stack = ExitStack()
                psb = _ps_stack.enter_context(tc.tile_pool(name="psb", bufs=1, space="PSUM"))
                lnp_g = wk.tile([P, D], F32, tag="lnpg", name="lnp_g")
                lnp_b = wk.tile([P, D], F32, tag="lnpb", name="lnp_b")
                prow = wk.tile([1, D], F32R, tag="prow", name="prow")

                def bcast_row(dst, row_dram_ap):
                    """dst[p, :] = row for all p (via K=1 matmul)."""
                    nc.sync.dma_start(prow[:], row_dram_ap)
                    pb = psb.tile([P, D], F32, tag="big", bufs=2, name="pb_bcast")
                    nc.tensor.matmul(pb[:, 0:512], ones_sb[:], prow[:, 0:512], start=True, stop=True)
                    nc.tensor.matmul(pb[:, 512:D], ones_sb[:], prow[:, 512:D], start=True, stop=True)
                    nc.vector.tensor_copy(out=dst[:], in_=pb[:])

                def layernorm(g_row, b_row):
                    """in-place LN over the feature axis of h."""
                    bcast_row(lnp_g, g_row)
                    bcast_row(lnp_b, b_row)
                    for qt in range(QT):
                        x = h[:, qt, :]
                        scr = wk.tile([P, D], F32, tag="scr", bufs=2, name="scr")
                        ssum = wk.tile([P, 1], F32, tag="st1", bufs=2, name="ssum")
                        nc.vector.tensor_reduce(out=ssum[:], in_=x, axis=AX, op=OP.add)
                        ssq = wk.tile([P, 1], F32, tag="st2", bufs=2, name="ssq")
                        nc.scalar.activation(scr[:], x, AF.Square, accum_out=ssq[:])
                        mean = wk.tile([P, 1], F32, tag="st3", bufs=2, name="mean")
                        nc.vector.tensor_scalar_mul(mean[:], ssum[:], 1.0 / D)
                        bias_t = wk.tile([P, 1], F32, tag="st4", bufs=2, name="bias_t")
                        nc.vector.tensor_tensor(bias_t[:], mean[:], mean[:], OP.mult)
                        nc.vector.tensor_scalar(bias_t[:], bias_t[:], -1.0, 1e-5, OP.mult, OP.add)
                        sstd = wk.tile([P, 1], F32, tag="st5", bufs=2, name="sstd")
                        nc.scalar.activation(sstd[:], ssq[:], AF.Sqrt, bias=bias_t[:], scale=1.0 / D)
                        rstd = wk.tile([P, 1], F32, tag="st6", bufs=2, name="rstd")
                        nc.vector.reciprocal(rstd[:], sstd[:])
                        nc.vector.tensor_scalar(scr[:], x, mean[:], rstd[:], OP.subtract, OP.mult)
                        nc.vector.tensor_tensor(scr[:], scr[:], lnp_g[:], OP.mult)
                        nc.vector.tensor_tensor(h[:, qt, :], scr[:], lnp_b[:], OP.add)

                def transpose_h(src_ap_fn, dst, ident=None, pdt=F32):
                    """dst[:, dt, qt*128:...] = transpose of 128x128 blocks of token-major src."""
                    if ident is None:
                        ident = id32
                    for qt in range(QT):
                        for dt in range(DT):
                            pt = psb.tile([P, 512], pdt, tag="sc", bufs=2, name="pt_tr")
                            nc.tensor.transpose(pt[:, :P], src_ap_fn(qt, dt), ident[:])
                            nc.vector.tensor_copy(out=dst[:, dt, qt * P : (qt + 1) * P], in_=pt[:, :P])

                for l in range(NL if not os.environ.get("TRN_SKIP_LAYERS") else 0):
                    with nc.named_scope(f"layer{l}"):
                        # --- h^T (bf16) for all projections ---
                        hT = wk.tile([P, DT, CH], BF16, tag="t6", bufs=2, name=f"hT_{l}")
                        transpose_h(lambda qt, dt: h[:, qt, dt * P : (dt + 1) * P], hT)

                        kv_in = dram.tile([KVROWS, KVW], BF16, name=f"kv_in{l}")

                        # --- K^T = (h @ wk)^T, scaled by 1/sqrt(hd) ---
                        kT_w = wk.tile([P, DT, CH], BF16, tag="t6", bufs=2, name=f"kT_{l}")
                        for op2 in range(3):  # pairs of 128-wide output chunks
                            wqs = wk.tile([P, DT, 256], BF16, tag="wqs", bufs=2, name="wk_c")
                            nc.sync.dma_start(
                                wqs[:],
                                wk_d[l].rearrange("(o p) f -> p o f", p=P)[:, :, op2 * 256 : (op2 + 1) * 256],
                            )
                            for j in range(2):
                                od = op2 * 2 + j
                                ps = psb.tile([P, 512], F32, tag="sc", bufs=2, name="ps_k")
                                for kt in range(DT):
                                    nc.tensor.matmul(
                                        ps[:], wqs[:, kt, j * P : (j + 1) * P], hT[:, kt, :],
                                        start=(kt == 0), stop=(kt == DT - 1),
                                    )
                                nc.vector.tensor_scalar_mul(kT_w[:, od, :], ps[:], HD ** (-0.5))
                        nc.sync.dma_start(
                            kv_in[0:D, 0:CH].rearrange("(o p) f -> p o f", p=P), kT_w[:]
                        )

                        # --- V = h @ wv (token-major, ones column per head) ---
                        wvf = wk.tile([P, DT, D], BF16, tag="wvf", name="wv_f")
                        nc.sync.dma_start(wvf[:], wv_d[l].rearrange("(o p) f -> p o f", p=P))
                        v_w = wk.tile([P, QT, H, HD + 1], BF16, tag="vw", name="v_w")
                        nc.vector.memset(v_w[:, :, :, HD], 1.0)
                        for qt in range(QT):
                            pv = psb.tile([P, D], F32, tag="big", bufs=2, name="ps_v")
                            for kt in range(DT):
                                nc.tensor.matmul(
                                    pv[:, 0:512],
                                    hT[:, kt, qt * P : (qt + 1) * P],
                                    wvf[:, kt, 0:512],
                                    start=(kt == 0),
                                    stop=(kt == DT - 1),
                                )
                                nc.tensor.matmul(
                                    pv[:, 512:D],
                                    hT[:, kt, qt * P : (qt + 1) * P],
                                    wvf[:, kt, 512:D],
                                    start=(kt == 0),
                                    stop=(kt == DT - 1),
                                )
                            nc.vector.tensor_copy(
                                out=v_w[:, qt, :, 0:HD],
                                in_=pv[:].rearrange("p (h e) -> p h e", e=HD),
                            )
                        nc.sync.dma_start(
                            kv_in[D:KVROWS, :].rearrange("(q p) (h e) -> p q h e", p=P, e=HD + 1),
                            v_w[:],
                        )

                        # --- all-gather packed K^T+V within each batch group ---
                        kv_ag = dram.tile([RANKS * KVROWS, KVW], BF16, name=f"kv_ag{l}")
                        nc.gpsimd.collective_compute(
                            "AllGather", OP.bypass, replica_groups=g4,
                            ins=[kv_in[:].opt()], outs=[kv_ag[:].opt()],
                        )

                        # --- Q^T = (h @ wq)^T (overlaps the all-gather) ---
                        qT = wk.tile([P, DT, CH], BF16, tag="q6", name=f"qT_{l}")
                        for op2 in range(3):
                            wqs = wk.tile([P, DT, 256], BF16, tag="wqs", bufs=2, name="wq_c")
                            nc.sync.dma_start(
                                wqs[:],
                                wq_d[l].rearrange("(o p) f -> p o f", p=P)[:, :, op2 * 256 : (op2 + 1) * 256],
                            )
                            for j in range(2):
                                od = op2 * 2 + j
                                ps = psb.tile([P, 512], F32, tag="sc", bufs=2, name="ps_q")
                                for kt in range(DT):
                                    nc.tensor.matmul(
                                        ps[:], wqs[:, kt, j * P : (j + 1) * P], hT[:, kt, :],
                                        start=(kt == 0), stop=(kt == DT - 1),
                                    )
                                nc.vector.tensor_copy(out=qT[:, od, :], in_=ps[:])

                        # --- attention ---
                        vag_sb = wk.tile([P, 16, KVW], BF16, tag="vag", name=f"vag_{l}")
                        o_acc = wk.tile([P, QT, H, HD + 1], F32, tag="oacc", name=f"oacc_{l}")
                        for r in range(RANKS):
                            ktc = wk.tile([P, DT, CH], BF16, tag="kts", bufs=2, name="ktc")
                            nc.sync.dma_start(
                                ktc[:],
                                kv_ag[KVROWS * r : KVROWS * r + D, 0:CH].rearrange(
                                    "(o p) f -> p o f", p=P
                                ),
                            )
                            nc.sync.dma_start(
                                vag_sb[:, 4 * r : 4 * (r + 1), :],
                                kv_ag[KVROWS * r + D : KVROWS * (r + 1), :].rearrange(
                                    "(o p) f -> p o f", p=P
                                ),
                            )
                            mks = wk.tile([P, QT, CH], BF16, tag="mks", bufs=2, name="mks")
                            nc.sync.dma_start(mks[:], masks_in[:, r * 4 : (r + 1) * 4, :])
                            for hh in range(H):
                                pb_ = (hh % 2) * 64
                                od = hh // 2
                                es = [None] * 4
                                for kt in range(4):
                                    ps_s = psb.tile([P, 512], F32, tag="sc", bufs=2, name="ps_s")
                                    nc.tensor.matmul(
                                        ps_s[:],
                                        ktc[pb_ : pb_ + 64, od, kt * P : (kt + 1) * P],
                                        qT[pb_ : pb_ + 64, od, :],
                                        start=True,
                                        stop=True,
                                    )
                                    e = wk.tile([P, CH], BF16, tag="es", bufs=8, name="es")
                                    nc.scalar.activation(e[:], ps_s[:], AF.Exp)
                                    nc.vector.tensor_tensor(e[:], e[:], mks[:, kt, :], OP.mult)
                                    es[kt] = e
                                for qt in range(QT):
                                    pav = psb.tile([P, HD + 1], F32, tag="avq", bufs=2, name="pav")
                                    for kt in range(4):
                                        nc.tensor.matmul(
                                            pav[:],
                                            es[kt][:, qt * P : (qt + 1) * P],
                                            vag_sb[:, r * 4 + kt, hh * (HD + 1) : (hh + 1) * (HD + 1)],
                                            start=(kt == 0),
                                            stop=(kt == 3),
                                        )
                                    if r == 0:
                                        nc.vector.tensor_copy(out=o_acc[:, qt, hh, :], in_=pav[:])
                                    else:
                                        nc.vector.tensor_tensor(
                                            o_acc[:, qt, hh, :], o_acc[:, qt, hh, :], pav[:], OP.add
                                        )
                        # normalize by the ones-column sums into a contiguous tile
                        o_nrm = wk.tile([P, QT, D], BF16, tag="onrm", name=f"onrm_{l}")
                        for qt in range(QT):
                            rec = wk.tile([P, H], F32, tag="rec", name="rec")
                            nc.vector.reciprocal(rec[:], o_acc[:, qt, :, HD])
                            nc.vector.tensor_tensor(
                                o_nrm[:, qt, :].rearrange("p (h e) -> p h e", e=HD),
                                o_acc[:, qt, :, 0:HD],
                                rec[:, :, None].to_broadcast([P, H, HD]),
                                OP.mult,
                            )

                        # --- O^T then mha = O @ wo, residual, LN1 ---
                        oT = wk.tile([P, DT, CH], BF16, tag="t6", bufs=2, name=f"oT_{l}")
                        transpose_h(
                            lambda qt, dt: o_nrm[:, qt, dt * P : (dt + 1) * P], oT, ident=idb, pdt=BF16
                        )
                        wof = wk.tile([P, DT, D], BF16, tag="wvf", name="wo_f")
                        nc.sync.dma_start(wof[:], wo_d[l].rearrange("(o p) f -> p o f", p=P))
                        for qt in range(QT):
                            pm = psb.tile([P, D], F32, tag="big", bufs=2, name="ps_wo")
                            for kt in range(DT):
                                nc.tensor.matmul(
                                    pm[:, 0:512],
                                    oT[:, kt, qt * P : (qt + 1) * P],
                                    wof[:, kt, 0:512],
                                    start=(kt == 0),
                                    stop=(kt == DT - 1),
                                )
                                nc.tensor.matmul(
                                    pm[:, 512:D],
                                    oT[:, kt, qt * P : (qt + 1) * P],
                                    wof[:, kt, 512:D],
                                    start=(kt == 0),
                                    stop=(kt == DT - 1),
                                )
                            nc.vector.tensor_tensor(h[:, qt, :], h[:, qt, :], pm[:], OP.add)
                        layernorm(ln1g_d[l : l + 1, :], ln1b_d[l : l + 1, :])

                        # --- FFN ---
                        hT2 = wk.tile([P, DT, CH], BF16, tag="t6", bufs=2, name=f"hT2_{l}")
                        transpose_h(lambda qt, dt: h[:, qt, dt * P : (dt + 1) * P], hT2)
                        b1_sb = wk.tile([P, FT], F32, tag="b1s", name="b1_sb")
                        nc.sync.dma_start(b1_sb[:], b1_d[l : l + 1, :].rearrange("a (o p) -> p (a o)", p=P))
                        b2_sb = wk.tile([1, D], BF16, tag="b2s", name="b2_sb")
                        nc.sync.dma_start(b2_sb[:], b2_d[l : l + 1, :])
                        for qp in range(2):
                            f1c = wk.tile([P, FT, 256], BF16, tag="f1c", name="f1c")
                            for dp in range(FT // 2):  # pairs of 128-wide dff chunks
                                w1c = wk.tile([P, DT, 256], BF16, tag="w1s", bufs=2, name="w1c")
                                nc.sync.dma_start(
                                    w1c[:],
                                    w1_d[l].rearrange("(o p) f -> p o f", p=P)[:, :, dp * 256 : (dp + 1) * 256],
                                )
                                for j in range(2):
                                    df = dp * 2 + j
                                    pf1 = psb.tile([P, 512], F32, tag="sc", bufs=2, name="ps_f1")
                                    for kt in range(DT):
                                        nc.tensor.matmul(
                                            pf1[:, 0:256],
                                            w1c[:, kt, j * P : (j + 1) * P],
                                            hT2[:, kt, qp * 256 : (qp + 1) * 256],
                                            start=(kt == 0),
                                            stop=(kt == DT - 1),
                                        )
                                    nc.scalar.activation(
                                        f1c[:, df, :], pf1[:, 0:256], AF.Relu, bias=b1_sb[:, df : df + 1]
                                    )
                            pf2 = [None, None]
                            for qtl in range(2):
                                pf2[qtl] = psb.tile([P, D], F32, tag="big", bufs=2, name="ps_f2")
                                nc.tensor.matmul(pf2[qtl][:, 0:512], ones_b[:], b2_sb[:, 0:512], start=True, stop=False)
                                nc.tensor.matmul(pf2[qtl][:, 512:D], ones_b[:], b2_sb[:, 512:D], start=True, stop=False)
                            for df in range(FT):
                                w2c = wk.tile([P, D], BF16, tag="w2s", bufs=2, name="w2c")
                                nc.sync.dma_start(
                                    w2c[:], w2_d[l].rearrange("(o p) f -> p o f", p=P)[:, df, :]
                                )
                                for qtl in range(2):
                                    nc.tensor.matmul(
                                        pf2[qtl][:, 0:512],
                                        f1c[:, df, qtl * P : (qtl + 1) * P],
                                        w2c[:, 0:512],
                                        start=False,
                                        stop=(df == FT - 1),
                                    )
                                    nc.tensor.matmul(
                                        pf2[qtl][:, 512:D],
                                        f1c[:, df, qtl * P : (qtl + 1) * P],
                                        w2c[:, 512:D],
                                        start=False,
                                        stop=(df == FT - 1),
                                    )
                            for qtl in range(2):
                                qt = qp * 2 + qtl
                                nc.vector.tensor_tensor(h[:, qt, :], h[:, qt, :], pf2[qtl][:], OP.add)
                        layernorm(ln2g_d[l : l + 1, :], ln2b_d[l : l + 1, :])

                # ---------- final LN + h^T (local only; LM head is token-sharded) ----------
                with nc.named_scope("final"):
                    layernorm(lnfg_d[:], lnfb_d[:])
                    hTf = wk.tile([P, DT, CH], BF16, tag="t6", bufs=2, name="hTf")
                    transpose_h(lambda qt, dt: h[:, qt, dt * P : (dt + 1) * P], hTf)

                # ---------- LM head: own 512 tokens x full (padded) vocab ----------
                _ps_stack.close()  # release the layer-loop PSUM banks
                if not os.environ.get("TRN_SKIP_LM"):
                    with (
                        tc.tile_pool(name="lmp", bufs=1) as lmp,
                        tc.tile_pool(name="pslm", bufs=1, space="PSUM") as pslm,
                        nc.named_scope("lmhead"),
                    ):
                        CW = 512  # one f32 PSUM bank per matmul
                        for n in range(NVC):
                            n0 = n * CW
                            lmw_c = lmp.tile([P, DT, CW], BF16, tag="lmw", bufs=2, name="lmw_c")
                            nc.sync.dma_start(
                                lmw_c[:],
                                lmw_d.rearrange("(o p) f -> p o f", p=P)[:, :, n0 : n0 + CW],
                            )
                            lmb_sb = lmp.tile([1, CW], BF16, tag="lmb", bufs=2, name="lmb_sb")
                            nc.sync.dma_start(lmb_sb[:], lmb_d[:, n0 : n0 + CW])
                            for qt in range(QT):
                                po = pslm.tile([P, CW], F32, tag="lmo", bufs=4, name="ps_lm")
                                nc.tensor.matmul(po[:], ones_b[:], lmb_sb[:], start=True, stop=False)
                                for dt in range(DT):
                                    nc.tensor.matmul(
                                        po[:],
                                        hTf[:, dt, qt * P : (qt + 1) * P],
                                        lmw_c[:, dt, :],
                                        start=False,
                                        stop=(dt == DT - 1),
                                    )
                                osb = lmp.tile([P, CW], BF16, tag="osb", bufs=4, name="o_sb")
                                nc.vector.tensor_copy(out=osb[:], in_=po[:])
                                nc.sync.dma_start(
                                    logits[qt * P : (qt + 1) * P, n0 : n0 + CW], osb[:]
                                )

    return _finish(nc)


def _finish(nc):
    nc.compile()
    return nc


def _pe_table():
    pos = np.arange(S, dtype=np.float32)[:, None]
    div = np.exp(np.arange(0, D, 2, dtype=np.float32) * (-math.log(10000.0) / D))
    pe = np.zeros((S, D), dtype=np.float32)
    pe[:, 0::2] = np.sin(pos * div)
    pe[:, 1::2] = np.cos(pos * div)
    return pe


def kernel(**inputs):
    if "nc" not in _CACHE:
        _CACHE["nc"] = build()
    nc = _CACHE["nc"]

    x = np.asarray(inputs["x"])
    f32 = lambda a: np.ascontiguousarray(np.asarray(a), dtype=np.float32)
    bf = lambda a: np.ascontiguousarray(np.asarray(a, dtype=np.float32).astype(ml_dtypes.bfloat16))
    # stack per-head projections into [D, H*HD]
    wq = bf(np.asarray(inputs["wq"], dtype=np.float32).transpose(0, 2, 1, 3).reshape(NL, D, D))
    wk_ = bf(np.asarray(inputs["wk"], dtype=np.float32).transpose(0, 2, 1, 3).reshape(NL, D, D))
    wv = bf(np.asarray(inputs["wv"], dtype=np.float32).transpose(0, 2, 1, 3).reshape(NL, D, D))
    pe = _pe_table()
    ones_c = np.ones((1, P), dtype=np.float32)

    lmw_pad = np.zeros((D, VP), dtype=ml_dtypes.bfloat16)
    lmw_pad[:, :V] = np.asarray(inputs["lm_w"], dtype=np.float32).astype(ml_dtypes.bfloat16)
    lmb_pad = np.zeros((1, VP), dtype=ml_dtypes.bfloat16)
    lmb_pad[0, :V] = np.asarray(inputs["lm_b"], dtype=np.float32).astype(ml_dtypes.bfloat16)

    common = {
        "tok_emb": bf(inputs["tok_emb"]),
        "wq": wq, "wk": wk_, "wv": wv, "wo": bf(inputs["wo"]),
        "w1": bf(inputs["w1"]), "w2": bf(inputs["w2"]),
        "b1": f32(inputs["b1"]), "b2": bf(inputs["b2"]),
        "ln1_g": f32(inputs["ln1_g"]), "ln1_b": f32(inputs["ln1_b"]),
        "ln2_g": f32(inputs["ln2_g"]), "ln2_b": f32(inputs["ln2_b"]),
        "lnf_g": f32(inputs["lnf_g"]).reshape(1, D),
        "lnf_b": f32(inputs["lnf_b"]).reshape(1, D),
        "lm_w": lmw_pad,
        "lm_b": lmb_pad,
        "c_ones": ones_c,
    }

    in_maps = []
    for c in range(NC):
        b, j = c // RANKS, c % RANKS
        toks = x[b, j * CH : (j + 1) * CH].astype(np.int32)  # [512]
        ids_c = toks.reshape(QT, P).T.copy()  # [128, 4]
        pe_c = pe[j * CH : (j + 1) * CH].reshape(QT, P, D).transpose(1, 0, 2).copy()
        kidx = np.arange(16 * P).reshape(16, P)  # [gkt, p] -> global k
        qidx = j * CH + np.arange(CH)  # [f] -> global q
        m = (kidx[None, :, :, None] <= qidx[None, None, None, :])  # [1,16,128,512]
        masks_c = m[0].transpose(1, 0, 2).astype(ml_dtypes.bfloat16)  # [128,16,512]
        in_maps.append({
            **common,
            "ids": ids_c,
            "pe": pe_c,
            "masks": np.ascontiguousarray(masks_c),
        })

    trace = bool(os.environ.get("TRN_KERNEL_TRACE"))
    res = run_bass_kernel_spmd(nc, in_maps, core_ids=list(range(NC)), trace=trace)
    _CACHE["last_result"] = res
    _CACHE["last_in_maps"] = in_maps
    out = np.concatenate(
        [np.asarray(res.results[c]["logits"], dtype=np.float32) for c in range(NC)], axis=0
    )
    return out[:, :V].reshape(B, S, V)


if __name__ == "__main__":
    import time

    t0 = time.time()
    nc = build()
    print(f"build ok: {time.time() - t0:.1f}s")



# revision 8
# speedup vs baseline: 1.0164x; 1.0164x over previous
"""Decoder-only transformer (GPT-style, post-LN) forward pass on 8 Trainium2 cores.

Sharding: tokens (batch*seq) are block-sharded 8 ways for the embedding and the
4 transformer layers (core c owns batch c//4, seq chunk c%4 of 512 tokens).
K^T and V are packed into one bf16 buffer and all-gathered per layer within
each batch's 4-core group (one collective per layer, 4 total). The LM head is
token-sharded: each core computes its own 512 tokens against the full vocab,
so no final all-gather is needed. Logits are written bf16 and upcast on host.

All weight/constant tensors are repacked on the host into the exact SBUF tile
layouts the kernel consumes, so every DMA is a contiguous per-partition block
(128 large descriptors) instead of hundreds of small strided ones.

Weights (wq/wk/wv/wo/w1/w2/lm_w/tok_emb) are shipped and consumed in bf16;
the residual stream h, layernorms, and all PSUM accumulation stay fp32.
lm_b is added on the host (exact), so the LM head needs no bias matmuls.
"""

import math
import os

import numpy as np
import ml_dtypes

import concourse.bass as bass
import concourse.bacc as bacc
import concourse.mybir as mybir
import concourse.tile as tile
from concourse.bass_utils import run_bass_kernel_spmd
from concourse.masks import make_identity

# model dims (hardcoded per problem spec)
V, S, D, NL, H = 50257, 2048, 768, 4, 12
HD, DF, B = 64, 3072, 2
NC = 8          # cores
CH = 512        # tokens per core
QT = 4          # 128-token tiles per core
DT = 6          # 128-wide d tiles
FT = 24         # 128-wide dff tiles
VP = 51200      # padded vocab (100 * 512)
NVC = 100       # vocab chunks of 512
RANKS = 4       # cores per batch group
KVW = 780       # H * (HD + 1)
KVC = DT * CH + QT * KVW   # 3072 + 3120 = 6192 bf16 per partition row

F32 = mybir.dt.float32
F32R = mybir.dt.float32r
BF16 = mybir.dt.bfloat16
I32 = mybir.dt.int32
AX = mybir.AxisListType.X
OP = mybir.AluOpType
AF = mybir.ActivationFunctionType
P = 128

_CACHE = {}


def build():
    nc = bacc.Bacc(None, target_bir_lowering=False, num_devices=NC)

    # ---- kernel I/O (all pre-packed host-side into SBUF-native layouts) ----
    ids = nc.dram_tensor("ids", [P, QT], I32, kind="ExternalInput")
    pe_in = nc.dram_tensor("pe", [P, QT, D], F32, kind="ExternalInput")
    masks_in = nc.dram_tensor("masks", [RANKS, P, QT, CH], BF16, kind="ExternalInput")
    tok_emb = nc.dram_tensor("tok_emb", [V, D], BF16, kind="ExternalInput")
    wq_d = nc.dram_tensor("wq", [NL, P, DT, D], BF16, kind="ExternalInput")
    wk_d = nc.dram_tensor("wk", [NL, P, DT, D], BF16, kind="ExternalInput")
    wv_d = nc.dram_tensor("wv", [NL, P, DT, D], BF16, kind="ExternalInput")
    wo_d = nc.dram_tensor("wo", [NL, P, DT, D], BF16, kind="ExternalInput")
    w1_d = nc.dram_tensor("w1", [NL * 12, P, DT, 256], BF16, kind="ExternalInput")
    w2_d = nc.dram_tensor("w2", [NL * FT, P, D], BF16, kind="ExternalInput")
    b1_d = nc.dram_tensor("b1", [NL, P, FT], F32, kind="ExternalInput")
    b2_d = nc.dram_tensor("b2", [NL, D], BF16, kind="ExternalInput")
    ln1g_d = nc.dram_tensor("ln1_g", [NL, D], F32R, kind="ExternalInput")
    ln1b_d = nc.dram_tensor("ln1_b", [NL, D], F32R, kind="ExternalInput")
    ln2g_d = nc.dram_tensor("ln2_g", [NL, D], F32R, kind="ExternalInput")
    ln2b_d = nc.dram_tensor("ln2_b", [NL, D], F32R, kind="ExternalInput")
    lnfg_d = nc.dram_tensor("lnf_g", [1, D], F32R, kind="ExternalInput")
    lnfb_d = nc.dram_tensor("lnf_b", [1, D], F32R, kind="ExternalInput")
    lmw_d = nc.dram_tensor("lm_w", [NVC, P, DT, 512], BF16, kind="ExternalInput")
    ones_d = nc.dram_tensor("c_ones", [1, P], F32R, kind="ExternalInput")
    # logits chunk (n, qt) lives at rows [(n*QT+qt)*128, +128) — host reassembles
    logits = nc.dram_tensor("logits", [NVC * QT * P, 512], BF16, kind="ExternalOutput")

    g4 = [[0, 1, 2, 3], [4, 5, 6, 7]]

    with tile.TileContext(nc) as tc:
        with (
            tc.tile_pool(name="pers", bufs=1) as pers,
            tc.tile_pool(name="dram", bufs=1, space="DRAM") as dram,
        ):
            h = pers.tile([P, QT, D], F32, name="h_res")
            ones_sb = pers.tile([1, P], F32R, name="ones_sb")
            nc.sync.dma_start(ones_sb[:], ones_d[:])
            ones_b = pers.tile([1, P], BF16, name="ones_b")
            nc.vector.memset(ones_b[:], 1.0)
            id32 = pers.tile([P, P], F32, name="id32")
            make_identity(nc, id32[:])
            idb = pers.tile([P, P], BF16, name="idb")
            make_identity(nc, idb[:])

            # ---------- embedding: gather + positional encoding ----------
            with tc.tile_pool(name="embp", bufs=1) as ep:
                ids_sb = ep.tile([P, QT], I32)
                nc.sync.dma_start(ids_sb[:], ids[:])
                pe_sb = ep.tile([P, QT, D], F32)
                nc.sync.dma_start(pe_sb[:], pe_in[:])
                for qt in range(QT):
                    emb = ep.tile([P, D], BF16, tag="emb", bufs=2)
                    nc.gpsimd.indirect_dma_start(
                        out=emb[:],
                        out_offset=None,
                        in_=tok_emb[:],
                        in_offset=bass.IndirectOffsetOnAxis(ap=ids_sb[:, qt : qt + 1], axis=0),
                    )
                    nc.vector.tensor_copy(out=h[:, qt, :], in_=emb[:])
                    nc.vector.tensor_tensor(h[:, qt, :], h[:, qt, :], pe_sb[:, qt, :], OP.add)

            # ---------- transformer layers ----------
            from contextlib import ExitStack

            with tc.tile_pool(name="wk", bufs=1) as wk:
                _ps_stack = ExitStack()
                psb = _ps_stack.enter_context(tc.tile_pool(name="psb", bufs=1, space="PSUM"))
                lnp_g = wk.tile([P, D], F32, tag="lnpg", name="lnp_g")
                lnp_b = wk.tile([P, D], F32, tag="lnpb", name="lnp_b")
                prow = wk.tile([1, D], F32R, tag="prow", name="prow")

                def bcast_row(dst, row_dram_ap):
                    """dst[p, :] = row for all p (via K=1 matmul)."""
                    nc.sync.dma_start(prow[:], row_dram_ap)
                    pb = psb.tile([P, D], F32, tag="big", bufs=2, name="pb_bcast")
                    nc.tensor.matmul(pb[:, 0:512], ones_sb[:], prow[:, 0:512], start=True, stop=True)
                    nc.tensor.matmul(pb[:, 512:D], ones_sb[:], prow[:, 512:D], start=True, stop=True)
                    nc.vector.tensor_copy(out=dst[:], in_=pb[:])

                def layernorm(g_row, b_row):
                    """in-place LN over the feature axis of h."""
                    bcast_row(lnp_g, g_row)
                    bcast_row(lnp_b, b_row)
                    for qt in range(QT):
                        x = h[:, qt, :]
                        scr = wk.tile([P, D], F32, tag="scr", bufs=2, name="scr")
                        ssum = wk.tile([P, 1], F32, tag="st1", bufs=2, name="ssum")
                        nc.vector.tensor_reduce(out=ssum[:], in_=x, axis=AX, op=OP.add)
                        ssq = wk.tile([P, 1], F32, tag="st2", bufs=2, name="ssq")
                        nc.scalar.activation(scr[:], x, AF.Square, accum_out=ssq[:])
                        mean = wk.tile([P, 1], F32, tag="st3", bufs=2, name="mean")
                        nc.vector.tensor_scalar_mul(mean[:], ssum[:], 1.0 / D)
                        bias_t = wk.tile([P, 1], F32, tag="st4", bufs=2, name="bias_t")
                        nc.vector.tensor_tensor(bias_t[:], mean[:], mean[:], OP.mult)
                        nc.vector.tensor_scalar(bias_t[:], bias_t[:], -1.0, 1e-5, OP.mult, OP.add)
                        sstd = wk.tile([P, 1], F32, tag="st5", bufs=2, name="sstd")
                        nc.scalar.activation(sstd[:], ssq[:], AF.Sqrt, bias=bias_t[:], scale=1.0 / D)
                        rstd = wk.tile([P, 1], F32, tag="st6", bufs=2, name="rstd")
                        nc.vector.reciprocal(rstd[:], sstd[:])
                        nc.vector.tensor_scalar(scr[:], x, mean[:], rstd[:], OP.subtract, OP.mult)
                        nc.vector.tensor_tensor(scr[:], scr[:], lnp_g[:], OP.mult)
                        nc.vector.tensor_tensor(h[:, qt, :], scr[:], lnp_b[:], OP.add)

                def transpose_h(src_ap_fn, dst, ident=None, pdt=F32):
                    """dst[:, dt, qt*128:...] = transpose of 128x128 blocks of token-major src."""
                    if ident is None:
                        ident = id32
                    for qt in range(QT):
                        for dt in range(DT):
                            pt = psb.tile([P, 512], pdt, tag="sc", bufs=2, name="pt_tr")
                            nc.tensor.transpose(pt[:, :P], src_ap_fn(qt, dt), ident[:])
                            nc.vector.tensor_copy(out=dst[:, dt, qt * P : (qt + 1) * P], in_=pt[:, :P])

                rep_layers = int(os.environ.get("TRN_REP_LAYERS", "1"))
                rep_coll = int(os.environ.get("TRN_REP_COLL", "1"))
                layer_seq = [] if os.environ.get("TRN_SKIP_LAYERS") else [
                    (l, r) for r in range(rep_layers) for l in range(NL)
                ]
                for l, _rep in layer_seq:
                    with nc.named_scope(f"layer{l}_{_rep}"):
                        # --- h^T (bf16) for all projections ---
                        hT = wk.tile([P, DT, CH], BF16, tag="t6", bufs=2, name=f"hT_{l}")
                        transpose_h(lambda qt, dt: h[:, qt, dt * P : (dt + 1) * P], hT)

                        kv_in = dram.tile([P, KVC], BF16, name=f"kv_in{l}_{_rep}")

                        # --- K^T = (h @ wk)^T, scaled by 1/sqrt(hd) ---
                        kT_w = wk.tile([P, DT, CH], BF16, tag="t6", bufs=2, name=f"kT_{l}")
                        wks = wk.tile([P, DT, D], BF16, tag="wqf", bufs=2, name="wk_f")
                        nc.sync.dma_start(wks[:], wk_d[l])
                        for od in range(DT):
                            ps = psb.tile([P, 512], F32, tag="sc", bufs=2, name="ps_k")
                            for kt in range(DT):
                                nc.tensor.matmul(
                                    ps[:], wks[:, kt, od * P : (od + 1) * P], hT[:, kt, :],
                                    start=(kt == 0), stop=(kt == DT - 1),
                                )
                            nc.vector.tensor_scalar_mul(kT_w[:, od, :], ps[:], HD ** (-0.5))
                        nc.sync.dma_start(
                            kv_in[:, 0 : DT * CH], kT_w[:].rearrange("p a b -> p (a b)")
                        )

                        # --- V = h @ wv (token-major, ones column per head) ---
                        wvf = wk.tile([P, DT, D], BF16, tag="wvf", bufs=2, name="wv_f")
                        nc.sync.dma_start(wvf[:], wv_d[l])
                        v_w = wk.tile([P, QT, H, HD + 1], BF16, tag="vw", name="v_w")
                        nc.vector.memset(v_w[:, :, :, HD], 1.0)
                        for qt in range(QT):
                            pv = psb.tile([P, D], F32, tag="big", bufs=2, name="ps_v")
                            for kt in range(DT):
                                nc.tensor.matmul(
                                    pv[:, 0:512],
                                    hT[:, kt, qt * P : (qt + 1) * P],
                                    wvf[:, kt, 0:512],
                                    start=(kt == 0),
                                    stop=(kt == DT - 1),
                                )
                                nc.tensor.matmul(
                                    pv[:, 512:D],
                                    hT[:, kt, qt * P : (qt + 1) * P],
                                    wvf[:, kt, 512:D],
                                    start=(kt == 0),
                                    stop=(kt == DT - 1),
                                )
                            nc.vector.tensor_copy(
                                out=v_w[:, qt, :, 0:HD],
                                in_=pv[:].rearrange("p (h e) -> p h e", e=HD),
                            )
                        nc.sync.dma_start(
                            kv_in[:, DT * CH : KVC], v_w[:].rearrange("p a b c -> p (a b c)")
                        )

                        # --- all-gather packed K^T+V within each batch group ---
                        kv_ag = dram.tile([RANKS * P, KVC], BF16, name=f"kv_ag{l}_{_rep}")
                        for _cr in range(rep_coll):
                            nc.gpsimd.collective_compute(
                                "AllGather", OP.bypass, replica_groups=g4,
                                ins=[kv_in[:].opt()], outs=[kv_ag[:].opt()],
                            )

                        # --- Q^T = (h @ wq)^T (overlaps the all-gather) ---
                        qT = wk.tile([P, DT, CH], BF16, tag="q6", name=f"qT_{l}")
                        wqs = wk.tile([P, DT, D], BF16, tag="wqf", bufs=2, name="wq_f")
                        nc.sync.dma_start(wqs[:], wq_d[l])
                        for od in range(DT):
                            ps = psb.tile([P, 512], F32, tag="sc", bufs=2, name="ps_q")
                            for kt in range(DT):
                                nc.tensor.matmul(
                                    ps[:], wqs[:, kt, od * P : (od + 1) * P], hT[:, kt, :],
                                    start=(kt == 0), stop=(kt == DT - 1),
                                )
                            nc.vector.tensor_copy(out=qT[:, od, :], in_=ps[:])

                        # --- attention ---
                        vag_sb = wk.tile([P, 16, KVW], BF16, tag="vag", name=f"vag_{l}")
                        o_acc = wk.tile([P, QT, H, HD + 1], F32, tag="oacc", name=f"oacc_{l}")
                        for r in range(RANKS):
                            ktc = wk.tile([P, DT, CH], BF16, tag="kts", bufs=2, name="ktc")
                            nc.sync.dma_start(
                                ktc[:],
                                kv_ag[r * P : (r + 1) * P, 0 : DT * CH].rearrange(
                                    "p (a b) -> p a b", b=CH
                                ),
                            )
                            nc.sync.dma_start(
                                vag_sb[:, 4 * r : 4 * (r + 1), :],
                                kv_ag[r * P : (r + 1) * P, DT * CH : KVC].rearrange(
                                    "p (a b) -> p a b", b=KVW
                                ),
                            )
                            mks = wk.tile([P, QT, CH], BF16, tag="mks", bufs=2, name="mks")
                            nc.sync.dma_start(mks[:], masks_in[r])
                            for hh in range(H):
                                pb_ = (hh % 2) * 64
                                od = hh // 2
                                es = [None] * 4
                                for kt in range(4):
                                    ps_s = psb.tile([P, 512], F32, tag="sc", bufs=2, name="ps_s")
                                    nc.tensor.matmul(
                                        ps_s[:],
                                        ktc[pb_ : pb_ + 64, od, kt * P : (kt + 1) * P],
                                        qT[pb_ : pb_ + 64, od, :],
                                        start=True,
                                        stop=True,
                                    )
                                    e = wk.tile([P, CH], BF16, tag="es", bufs=8, name="es")
                                    nc.scalar.activation(e[:], ps_s[:], AF.Exp)
                                    nc.vector.tensor_tensor(e[:], e[:], mks[:, kt, :], OP.mult)
                                    es[kt] = e
                                for qt in range(QT):
                                    pav = psb.tile([P, HD + 1], F32, tag="avq", bufs=2, name="pav")
                                    for kt in range(4):
                                        nc.tensor.matmul(
                                            pav[:],
                                            es[kt][:, qt * P : (qt + 1) * P],
                                            vag_sb[:, r * 4 + kt, hh * (HD + 1) : (hh + 1) * (HD + 1)],
                                            start=(kt == 0),
                                            stop=(kt == 3),
                                        )
                                    if r == 0:
                                        nc.vector.tensor_copy(out=o_acc[:, qt, hh, :], in_=pav[:])
                                    else:
                                        nc.vector.tensor_tensor(
                                            o_acc[:, qt, hh, :], o_acc[:, qt, hh, :], pav[:], OP.add
                                        )
                        # normalize by the ones-column sums into a contiguous tile
                        o_nrm = wk.tile([P, QT, D], BF16, tag="onrm", name=f"onrm_{l}")
                        for qt in range(QT):
                            rec = wk.tile([P, H], F32, tag="rec", name="rec")
                            nc.vector.reciprocal(rec[:], o_acc[:, qt, :, HD])
                            nc.vector.tensor_tensor(
                                o_nrm[:, qt, :].rearrange("p (h e) -> p h e", e=HD),
                                o_acc[:, qt, :, 0:HD],
                                rec[:, :, None].to_broadcast([P, H, HD]),
                                OP.mult,
                            )

                        # --- O^T then mha = O @ wo, residual, LN1 ---
                        oT = wk.tile([P, DT, CH], BF16, tag="t6", bufs=2, name=f"oT_{l}")
                        transpose_h(
                            lambda qt, dt: o_nrm[:, qt, dt * P : (dt + 1) * P], oT, ident=idb, pdt=BF16
                        )
                        wof = wk.tile([P, DT, D], BF16, tag="wvf", bufs=2, name="wo_f")
                        nc.sync.dma_start(wof[:], wo_d[l])
                        for qt in range(QT):
                            pm = psb.tile([P, D], F32, tag="big", bufs=2, name="ps_wo")
                            for kt in range(DT):
                                nc.tensor.matmul(
                                    pm[:, 0:512],
                                    oT[:, kt, qt * P : (qt + 1) * P],
                                    wof[:, kt, 0:512],
                                    start=(kt == 0),
                                    stop=(kt == DT - 1),
                                )
                                nc.tensor.matmul(
                                    pm[:, 512:D],
                                    oT[:, kt, qt * P : (qt + 1) * P],
                                    wof[:, kt, 512:D],
                                    start=(kt == 0),
                                    stop=(kt == DT - 1),
                                )
                            nc.vector.tensor_tensor(h[:, qt, :], h[:, qt, :], pm[:], OP.add)
                        layernorm(ln1g_d[l : l + 1, :], ln1b_d[l : l + 1, :])

                        # --- FFN ---
                        hT2 = wk.tile([P, DT, CH], BF16, tag="t6", bufs=2, name=f"hT2_{l}")
                        transpose_h(lambda qt, dt: h[:, qt, dt * P : (dt + 1) * P], hT2)
                        b1_sb = wk.tile([P, FT], F32, tag="b1s", name="b1_sb")
                        nc.sync.dma_start(b1_sb[:], b1_d[l])
                        b2_sb = wk.tile([1, D], BF16, tag="b2s", name="b2_sb")
                        nc.sync.dma_start(b2_sb[:], b2_d[l : l + 1, :])
                        for qp in range(2):
                            f1c = wk.tile([P, FT, 256], BF16, tag="f1c", name="f1c")
                            for dp in range(FT // 2):  # pairs of 128-wide dff chunks
                                w1c = wk.tile([P, DT, 256], BF16, tag="w1s", bufs=2, name="w1c")
                                nc.sync.dma_start(w1c[:], w1_d[l * 12 + dp])
                                for j in range(2):
                                    df = dp * 2 + j
                                    pf1 = psb.tile([P, 512], F32, tag="sc", bufs=2, name="ps_f1")
                                    for kt in range(DT):
                                        nc.tensor.matmul(
                                            pf1[:, 0:256],
                                            w1c[:, kt, j * P : (j + 1) * P],
                                            hT2[:, kt, qp * 256 : (qp + 1) * 256],
                                            start=(kt == 0),
                                            stop=(kt == DT - 1),
                                        )
                                    nc.scalar.activation(
                                        f1c[:, df, :], pf1[:, 0:256], AF.Relu, bias=b1_sb[:, df : df + 1]
                                    )
                            pf2 = [None, None]
                            for qtl in range(2):
                                pf2[qtl] = psb.tile([P, D], F32, tag="big", bufs=2, name="ps_f2")
                                nc.tensor.matmul(pf2[qtl][:, 0:512], ones_b[:], b2_sb[:, 0:512], start=True, stop=False)
                                nc.tensor.matmul(pf2[qtl][:, 512:D], ones_b[:], b2_sb[:, 512:D], start=True, stop=False)
                            for df in range(FT):
                                w2c = wk.tile([P, D], BF16, tag="w2s", bufs=2, name="w2c")
                                nc.sync.dma_start(w2c[:], w2_d[l * FT + df])
                                for qtl in range(2):
                                    nc.tensor.matmul(
                                        pf2[qtl][:, 0:512],
                                        f1c[:, df, qtl * P : (qtl + 1) * P],
                                        w2c[:, 0:512],
                                        start=False,
                                        stop=(df == FT - 1),
                                    )
                                    nc.tensor.matmul(
                                        pf2[qtl][:, 512:D],
                                        f1c[:, df, qtl * P : (qtl + 1) * P],
                                        w2c[:, 512:D],
                                        start=False,
                                        stop=(df == FT - 1),
                                    )
                            for qtl in range(2):
                                qt = qp * 2 + qtl
                                nc.vector.tensor_tensor(h[:, qt, :], h[:, qt, :], pf2[qtl][:], OP.add)
                        layernorm(ln2g_d[l : l + 1, :], ln2b_d[l : l + 1, :])

                # ---------- final LN + h^T (local only; LM head is token-sharded) ----------
                with nc.named_scope("final"):
                    layernorm(lnfg_d[:], lnfb_d[:])
                    hTf = wk.tile([P, DT, CH], BF16, tag="t6", bufs=2, name="hTf")
                    transpose_h(lambda qt, dt: h[:, qt, dt * P : (dt + 1) * P], hTf)

                # ---------- LM head: own 512 tokens x full (padded) vocab ----------
                _ps_stack.close()  # release the layer-loop PSUM banks
                if not os.environ.get("TRN_SKIP_LM"):
                    with (
                        tc.tile_pool(name="lmp", bufs=1) as lmp,
                        tc.tile_pool(name="pslm", bufs=1, space="PSUM") as pslm,
                        nc.named_scope("lmhead"),
                    ):
                        CW = 512  # one f32 PSUM bank per matmul
                        rep_lm = int(os.environ.get("TRN_REP_LM", "1"))
                        for nn_ in range(NVC * rep_lm):
                            n = nn_ % NVC
                            lmw_c = lmp.tile([P, DT, CW], BF16, tag="lmw", bufs=3, name="lmw_c")
                            nc.sync.dma_start(lmw_c[:], lmw_d[n])
                            for qt in range(QT):
                                po = pslm.tile([P, CW], F32, tag="lmo", bufs=4, name="ps_lm")
                                for dt in range(DT):
                                    nc.tensor.matmul(
                                        po[:],
                                        hTf[:, dt, qt * P : (qt + 1) * P],
                                        lmw_c[:, dt, :],
                                        start=(dt == 0),
                                        stop=(dt == DT - 1),
                                    )
                                osb = lmp.tile([P, CW], BF16, tag="osb", bufs=4, name="o_sb")
                                nc.vector.tensor_copy(out=osb[:], in_=po[:])
                                nc.sync.dma_start(
                                    logits[(n * QT + qt) * P : (n * QT + qt + 1) * P, :], osb[:]
                                )

    return _finish(nc)


def _finish(nc):
    nc.compile()
    return nc


def _pe_table():
    pos = np.arange(S, dtype=np.float32)[:, None]
    div = np.exp(np.arange(0, D, 2, dtype=np.float32) * (-math.log(10000.0) / D))
    pe = np.zeros((S, D), dtype=np.float32)
    pe[:, 0::2] = np.sin(pos * div)
    pe[:, 1::2] = np.cos(pos * div)
    return pe


def prep_in_maps(inputs):
    """Host-side preprocessing: cast, transpose, and repack every tensor into
    the kernel's SBUF-native layouts."""
    x = np.asarray(inputs["x"])
    f32 = lambda a: np.ascontiguousarray(np.asarray(a), dtype=np.float32)
    bf = lambda a: np.ascontiguousarray(np.asarray(a, dtype=np.float32).astype(ml_dtypes.bfloat16))

    def pack_po(w):
        # [rows, cols] with rows = T*128 -> [128, T, cols]
        w = np.asarray(w, dtype=np.float32)
        t = w.shape[0] // P
        return np.ascontiguousarray(
            w.reshape(t, P, w.shape[1]).transpose(1, 0, 2).astype(ml_dtypes.bfloat16)
        )

    # stack per-head projections into [D, H*HD], then partition-major
    def pack_qkv(w):
        w = np.asarray(w, dtype=np.float32).transpose(0, 2, 1, 3).reshape(NL, D, D)
        return np.stack([pack_po(w[i]) for i in range(NL)])

    wq = pack_qkv(inputs["wq"])
    wk_ = pack_qkv(inputs["wk"])
    wv = pack_qkv(inputs["wv"])
    wo = np.stack([pack_po(np.asarray(inputs["wo"], dtype=np.float32)[i]) for i in range(NL)])

    # w1 -> [NL*12, 128, DT, 256] (dp chunks of 256 dff cols)
    w1f = np.asarray(inputs["w1"], dtype=np.float32)  # [NL, D, DF]
    w1p = w1f.reshape(NL, DT, P, 12, 256).transpose(0, 3, 2, 1, 4)  # [NL, 12, 128, DT, 256]
    w1p = np.ascontiguousarray(w1p.reshape(NL * 12, P, DT, 256).astype(ml_dtypes.bfloat16))
    # w2 -> [NL*FT, 128, D] (partition over dff)
    w2f = np.asarray(inputs["w2"], dtype=np.float32)  # [NL, DF, D]
    w2p = w2f.reshape(NL, FT, P, D)
    w2p = np.ascontiguousarray(w2p.reshape(NL * FT, P, D).astype(ml_dtypes.bfloat16))
    # b1 -> [NL, 128, FT]
    b1p = np.ascontiguousarray(
        np.asarray(inputs["b1"], dtype=np.float32).reshape(NL, FT, P).transpose(0, 2, 1)
    )
    # lm_w -> [NVC, 128, DT, 512] (zero-padded vocab)
    lmw = np.zeros((D, VP), dtype=np.float32)
    lmw[:, :V] = np.asarray(inputs["lm_w"], dtype=np.float32)
    lmw_pack = np.ascontiguousarray(
        lmw.reshape(DT, P, NVC, 512).transpose(2, 1, 0, 3).astype(ml_dtypes.bfloat16)
    )

    pe = _pe_table()
    ones_c = np.ones((1, P), dtype=np.float32)

    common = {
        "tok_emb": bf(inputs["tok_emb"]),
        "wq": wq, "wk": wk_, "wv": wv, "wo": wo,
        "w1": w1p, "w2": w2p,
        "b1": b1p, "b2": bf(inputs["b2"]),
        "ln1_g": f32(inputs["ln1_g"]), "ln1_b": f32(inputs["ln1_b"]),
        "ln2_g": f32(inputs["ln2_g"]), "ln2_b": f32(inputs["ln2_b"]),
        "lnf_g": f32(inputs["lnf_g"]).reshape(1, D),
        "lnf_b": f32(inputs["lnf_b"]).reshape(1, D),
        "lm_w": lmw_pack,
        "c_ones": ones_c,
    }

    in_maps = []
    for c in range(NC):
        b, j = c // RANKS, c % RANKS
        toks = x[b, j * CH : (j + 1) * CH].astype(np.int32)  # [512]
        ids_c = toks.reshape(QT, P).T.copy()  # [128, 4]
        pe_c = pe[j * CH : (j + 1) * CH].reshape(QT, P, D).transpose(1, 0, 2).copy()
        kidx = np.arange(16 * P).reshape(16, P)  # [gkt, p] -> global k
        qidx = j * CH + np.arange(CH)  # [f] -> global q
        m = (kidx[:, :, None] <= qidx[None, None, :])  # [16,128,512]
        masks_c = m.reshape(RANKS, QT, P, CH).transpose(0, 2, 1, 3).astype(ml_dtypes.bfloat16)
        in_maps.append({
            **common,
            "ids": ids_c,
            "pe": pe_c,
            "masks": np.ascontiguousarray(masks_c),
        })
    return in_maps


def postprocess(res, inputs):
    """[NVC*QT*128, 512] bf16 per core -> [B, S, V] f32 (+ lm_b on host)."""
    outs = []
    for c in range(NC):
        a = np.asarray(res[c]["logits"], dtype=np.float32)
        a = a.reshape(NVC, QT, P, 512).transpose(1, 2, 0, 3).reshape(CH, VP)
        outs.append(a)
    out = np.concatenate(outs, axis=0)[:, :V].reshape(B, S, V)
    lm_b = np.asarray(inputs["lm_b"], dtype=np.float32)
    if lm_b.any():
        out += lm_b
    return out


def kernel(**inputs):
    if "nc" not in _CACHE:
        _CACHE["nc"] = build()
    nc = _CACHE["nc"]

    in_maps = prep_in_maps(inputs)

    trace = bool(os.environ.get("TRN_KERNEL_TRACE"))
    res = run_bass_kernel_spmd(nc, in_maps, core_ids=list(range(NC)), trace=trace)
    _CACHE["last_result"] = res
    _CACHE["last_in_maps"] = in_maps
    return postprocess(res.results, inputs)


if __name__ == "__main__":
    import time

    t0 = time.time()
    nc = build()
    print(f"build ok: {time.time() - t0:.1f}s")


# revision 20
# speedup vs baseline: 1.5243x; 1.4997x over previous
"""Decoder-only transformer (GPT-style, post-LN) forward pass on 8 Trainium2 cores.

Sharding: tokens are STRIDE-sharded across the 4 cores of each batch group
(core j of batch b owns tokens {4i+j}), which balances causal attention:
every q-tile t only attends k-tiles s <= t, cutting attention work to 10/16
of the dense version with an identical program on every core.

The residual stream h lives d-major ("transposed", [128 d-part, 6 d-tiles,
512 tokens]) for the whole kernel, so no per-layer transposes are needed:
projections, FFN, layernorm (via ones-matmul column reductions) and the LM
head all consume it directly. Attention produces o^T straight from PSUM
(lhsT = V token-major, rhs = exp-scores [k, q]), accumulating all 16 (rank,
s-tile) contributions per head in a single PSUM bank, with a ones-column
carrying the softmax denominators.

K^T and V are packed into one bf16 buffer and all-gathered per layer within
each batch's 4-core group (one collective per layer). The LM head is
token-sharded (each core: own 512 tokens x full padded vocab). The embedding
gather + positional encoding run on the host, which ships h0 directly.
All weights are host-repacked so every DMA is contiguous per partition.
lm_b is added on the host (exact). Residual h stays fp32; matmul inputs bf16.
"""

import math
import os

import numpy as np
import ml_dtypes

import concourse.bass as bass
import concourse.bacc as bacc
import concourse.mybir as mybir
import concourse.tile as tile
from concourse.bass_utils import run_bass_kernel_spmd

# model dims (hardcoded per problem spec)
V, S, D, NL, H = 50257, 2048, 768, 4, 12
HD, DF, B = 64, 3072, 2
NC = 8          # cores
CH = 512        # tokens per core
QT = 4          # 128-token tiles per core
DT = 6          # 128-wide d tiles
FT = 24         # 128-wide dff tiles
VP = 51200      # padded vocab (100 * 512)
NVC = 100       # vocab chunks of 512
RANKS = 4       # cores per batch group
KVW = 780       # H * (HD + 1)
KVC = DT * CH + QT * KVW   # 3072 + 3120 = 6192 bf16 per partition row
NLN = 2 * NL + 1           # layernorm param sets (2 per layer + final)

F32 = mybir.dt.float32
BF16 = mybir.dt.bfloat16
AX = mybir.AxisListType.X
OP = mybir.AluOpType
AF = mybir.ActivationFunctionType
P = 128

_CACHE = {}


def build():
    nc = bacc.Bacc(None, target_bir_lowering=False, num_devices=NC)

    # ---- kernel I/O (all pre-packed host-side into SBUF-native layouts) ----
    h0_d = nc.dram_tensor("h0", [P, DT, CH], F32, kind="ExternalInput")
    mdiag_d = nc.dram_tensor("mdiag", [P, RANKS, P], BF16, kind="ExternalInput")
    lnp_d = nc.dram_tensor("lnp", [P, NLN * 2 * DT], F32, kind="ExternalInput")
    wq_d = nc.dram_tensor("wq", [NL, P, DT, D], BF16, kind="ExternalInput")
    wk_d = nc.dram_tensor("wk", [NL, P, DT, D], BF16, kind="ExternalInput")
    wv_d = nc.dram_tensor("wv", [NL, P, DT, D], BF16, kind="ExternalInput")
    wo_d = nc.dram_tensor("wo", [NL, P, DT, D], BF16, kind="ExternalInput")
    w1_d = nc.dram_tensor("w1", [NL * 12, P, DT, 256], BF16, kind="ExternalInput")
    w2_d = nc.dram_tensor("w2", [NL * FT, P, D], BF16, kind="ExternalInput")
    b1_d = nc.dram_tensor("b1", [NL, P, FT], F32, kind="ExternalInput")
    b2_d = nc.dram_tensor("b2", [NL, D], BF16, kind="ExternalInput")
    lmw_d = nc.dram_tensor("lm_w", [NVC, P, DT, 512], BF16, kind="ExternalInput")
    # logits chunk (n, qt) lives at rows [(n*QT+qt)*128, +128) — host reassembles
    logits = nc.dram_tensor("logits", [NVC * QT * P, 512], BF16, kind="ExternalOutput")
    DBG = bool(os.environ.get("TRN_DEBUG"))
    if DBG:
        dbg_kT = nc.dram_tensor("dbg_kT", [P, DT, CH], BF16, kind="ExternalOutput")
        dbg_qT = nc.dram_tensor("dbg_qT", [P, DT, CH], BF16, kind="ExternalOutput")
        dbg_vw = nc.dram_tensor("dbg_vw", [P, QT * H * (HD + 1)], BF16, kind="ExternalOutput")
        dbg_oall = nc.dram_tensor("dbg_oall", [P, DT, CH], BF16, kind="ExternalOutput")
        dbg_h1 = nc.dram_tensor("dbg_h1", [P, DT, CH], F32, kind="ExternalOutput")
        dbg_h2 = nc.dram_tensor("dbg_h2", [P, DT, CH], F32, kind="ExternalOutput")

    g4 = [[0, 1, 2, 3], [4, 5, 6, 7]]
    HG = 6  # heads per attention group (bounded by PSUM banks)

    with tile.TileContext(nc) as tc:
        with (
            tc.tile_pool(name="pers", bufs=1) as pers,
            tc.tile_pool(name="dram", bufs=1, space="DRAM") as dram,
            tc.tile_pool(name="wk", bufs=1) as wk,
        ):
            h = pers.tile([P, DT, CH], F32, name="h_res")
            nc.sync.dma_start(h[:], h0_d[:])
            hb = pers.tile([P, DT, CH], BF16, name="hb")
            mdiag = pers.tile([P, RANKS, P], BF16, name="mdiag")
            nc.sync.dma_start(mdiag[:], mdiag_d[:])
            lnp = pers.tile([P, NLN * 2 * DT], F32, name="lnp")
            nc.sync.dma_start(lnp[:], lnp_d[:])
            ones_row = pers.tile([1, CH], BF16, name="ones_row")
            nc.vector.memset(ones_row[:], 1.0)
            ones_col = pers.tile([P, 1], BF16, name="ones_col")
            nc.vector.memset(ones_col[:], 1.0)
            onesf = pers.tile([1, P], F32, name="onesf")
            nc.vector.memset(onesf[:], 1.0)
            eps_t = pers.tile([1, 1], F32, name="eps_t")
            nc.vector.memset(eps_t[:], 1e-5)

            _psb_ctx = tc.tile_pool(name="psb", bufs=1, space="PSUM")
            psb = _psb_ctx.__enter__()

            def refresh_hb():
                for dt in range(DT):
                    nc.vector.tensor_copy(out=hb[:, dt, :], in_=h[:, dt, :])

            refresh_hb()

            def layernorm_d(ln_idx):
                """in-place LN of h over the d axis (stats via ones-matmuls)."""
                refresh_hb()  # stats must see the post-residual h
                ps_m = psb.tile([1, CH], F32, tag="sc", bufs=2, name="ps_m")
                ps_q = psb.tile([1, CH], F32, tag="sc", bufs=2, name="ps_q")
                for dt in range(DT):
                    nc.tensor.matmul(ps_m[:], ones_col[:], hb[:, dt, :],
                                     start=(dt == 0), stop=(dt == DT - 1))
                for dt in range(DT):
                    sq = wk.tile([P, CH], BF16, tag="sq", bufs=2, name="sq")
                    nc.scalar.activation(sq[:], hb[:, dt, :], AF.Square)
                    nc.tensor.matmul(ps_q[:], ones_col[:], sq[:],
                                     start=(dt == 0), stop=(dt == DT - 1))
                mean = wk.tile([1, CH], F32, tag="lns", bufs=4, name="mean")
                nc.vector.tensor_scalar_mul(mean[:], ps_m[:], 1.0 / D)
                var = wk.tile([1, CH], F32, tag="lns", bufs=4, name="var")
                nc.vector.tensor_tensor(var[:], mean[:], mean[:], OP.mult)
                ex2 = wk.tile([1, CH], F32, tag="lns", bufs=4, name="ex2")
                nc.vector.tensor_scalar_mul(ex2[:], ps_q[:], 1.0 / D)
                nc.vector.tensor_tensor(var[:], ex2[:], var[:], OP.subtract)
                sstd = wk.tile([1, CH], F32, tag="lns", bufs=4, name="sstd")
                nc.scalar.activation(sstd[:], var[:], AF.Sqrt, bias=eps_t[:])
                rstd = wk.tile([1, CH], F32, tag="lns", bufs=4, name="rstd")
                nc.vector.reciprocal(rstd[:], sstd[:])
                mr = wk.tile([1, CH], F32, tag="lns", bufs=4, name="mr")
                nc.vector.tensor_tensor(mr[:], mean[:], rstd[:], OP.mult)
                c1 = psb.tile([P, CH], F32, tag="sc", bufs=2, name="c1")
                nc.tensor.matmul(c1[:], onesf[:], rstd[:], start=True, stop=True)
                c2 = psb.tile([P, CH], F32, tag="sc", bufs=2, name="c2")
                nc.tensor.matmul(c2[:], onesf[:], mr[:], start=True, stop=True)
                gcol = lambda dt: lnp[:, (ln_idx * 2 + 0) * DT + dt : (ln_idx * 2 + 0) * DT + dt + 1]
                bcol = lambda dt: lnp[:, (ln_idx * 2 + 1) * DT + dt : (ln_idx * 2 + 1) * DT + dt + 1]
                for dt in range(DT):
                    tmp = wk.tile([P, CH], F32, tag="scr", bufs=2, name="lntmp")
                    nc.vector.tensor_tensor(tmp[:], h[:, dt, :], c1[:], OP.mult)
                    nc.vector.tensor_tensor(tmp[:], tmp[:], c2[:], OP.subtract)
                    nc.vector.tensor_scalar(h[:, dt, :], tmp[:], gcol(dt), bcol(dt), OP.mult, OP.add)
                refresh_hb()

            rep_layers = int(os.environ.get("TRN_REP_LAYERS", "1"))
            rep_coll = int(os.environ.get("TRN_REP_COLL", "1"))
            layer_seq = [] if os.environ.get("TRN_SKIP_LAYERS") else [
                (l, r) for r in range(rep_layers) for l in range(NL)
            ]
            for l, _rep in layer_seq:
                with nc.named_scope(f"layer{l}_{_rep}"):
                    kv_in = dram.tile([P, KVC], BF16, name=f"kv_in{l}_{_rep}")

                    # --- K^T = (h @ wk)^T scaled by 1/sqrt(hd), d-major ---
                    wks = wk.tile([P, DT, D], BF16, tag="wqf", bufs=2, name="wk_f")
                    nc.sync.dma_start(wks[:], wk_d[l])
                    kT_w = wk.tile([P, DT, CH], BF16, tag="kTw", name="kT_w")
                    for od in range(DT):
                        ps = psb.tile([P, CH], F32, tag="sc", bufs=2, name="ps_k")
                        for dt in range(DT):
                            nc.tensor.matmul(
                                ps[:], wks[:, dt, od * P : (od + 1) * P], hb[:, dt, :],
                                start=(dt == 0), stop=(dt == DT - 1),
                            )
                        nc.vector.tensor_scalar_mul(kT_w[:, od, :], ps[:], HD ** (-0.5))
                    nc.sync.dma_start(kv_in[:, 0 : DT * CH], kT_w[:].rearrange("p a b -> p (a b)"))

                    # --- V = h @ wv (token-major, ones column per head) ---
                    wvf = wk.tile([P, DT, D], BF16, tag="wvf", bufs=2, name="wv_f")
                    nc.sync.dma_start(wvf[:], wv_d[l])
                    v_w = wk.tile([P, QT, H, HD + 1], BF16, tag="vw", name="v_w")
                    nc.vector.memset(v_w[:, :, :, HD], 1.0)
                    for qt in range(QT):
                        pva = psb.tile([P, CH], F32, tag="sc", bufs=2, name="ps_va")
                        pvb = psb.tile([P, 256], F32, tag="sc", bufs=2, name="ps_vb")
                        for dt in range(DT):
                            nc.tensor.matmul(
                                pva[:], hb[:, dt, qt * P : (qt + 1) * P], wvf[:, dt, 0:512],
                                start=(dt == 0), stop=(dt == DT - 1),
                            )
                            nc.tensor.matmul(
                                pvb[:], hb[:, dt, qt * P : (qt + 1) * P], wvf[:, dt, 512:D],
                                start=(dt == 0), stop=(dt == DT - 1),
                            )
                        nc.vector.tensor_copy(
                            out=v_w[:, qt, 0:8, 0:HD],
                            in_=pva[:].rearrange("p (h e) -> p h e", e=HD),
                        )
                        nc.vector.tensor_copy(
                            out=v_w[:, qt, 8:H, 0:HD],
                            in_=pvb[:].rearrange("p (h e) -> p h e", e=HD),
                        )
                    nc.sync.dma_start(kv_in[:, DT * CH : KVC], v_w[:].rearrange("p a b c -> p (a b c)"))

                    # --- all-gather packed K^T+V within each batch group ---
                    kv_ag = dram.tile([RANKS * P, KVC], BF16, name=f"kv_ag{l}_{_rep}")
                    for _cr in range(rep_coll):
                        nc.gpsimd.collective_compute(
                            "AllGather", OP.bypass, replica_groups=g4,
                            ins=[kv_in[:].opt()], outs=[kv_ag[:].opt()],
                        )

                    # --- Q^T (overlaps the all-gather) ---
                    wqs = wk.tile([P, DT, D], BF16, tag="wqf", bufs=2, name="wq_f")
                    nc.sync.dma_start(wqs[:], wq_d[l])
                    qT = wk.tile([P, DT, CH], BF16, tag="q6", name="qT")
                    for od in range(DT):
                        ps = psb.tile([P, CH], F32, tag="sc", bufs=2, name="ps_q")
                        for dt in range(DT):
                            nc.tensor.matmul(
                                ps[:], wqs[:, dt, od * P : (od + 1) * P], hb[:, dt, :],
                                start=(dt == 0), stop=(dt == DT - 1),
                            )
                        nc.vector.tensor_copy(out=qT[:, od, :], in_=ps[:])

                    # --- attention: causal-skip, o^T accumulated per head in PSUM ---
                    o_all = wk.tile([P, DT, CH], BF16, tag="oall", name="o_all")
                    for hg in range(H // HG):
                        oT_ps = [None] * HG
                        for r in range(RANKS):
                            ktc = wk.tile([P, DT, CH], BF16, tag="kts", bufs=2, name="ktc")
                            nc.sync.dma_start(
                                ktc[:],
                                kv_ag[r * P : (r + 1) * P, 0 : DT * CH].rearrange(
                                    "p (a b) -> p a b", b=CH
                                ),
                            )
                            vag = wk.tile([P, QT, KVW], BF16, tag="vga", bufs=2, name="vag")
                            nc.sync.dma_start(
                                vag[:],
                                kv_ag[r * P : (r + 1) * P, DT * CH : KVC].rearrange(
                                    "p (a b) -> p a b", b=KVW
                                ),
                            )
                            for hx in range(HG):
                                hh = hg * HG + hx
                                pb_ = (hh % 2) * 64
                                od = hh // 2
                                if r == 0:
                                    oT_ps[hx] = psb.tile([HD + 1, CH], F32, tag="oT", bufs=HG, name="oT_ps")
                                for s_ in range(QT):
                                    N = CH - s_ * P
                                    ps_s = psb.tile([P, CH], F32, tag="sc", bufs=2, name="ps_s")
                                    nc.tensor.matmul(
                                        ps_s[:, 0:N],
                                        ktc[pb_ : pb_ + 64, od, s_ * P : (s_ + 1) * P],
                                        qT[pb_ : pb_ + 64, od, s_ * P : CH],
                                        start=True, stop=True,
                                    )
                                    e = wk.tile([P, CH], BF16, tag="es", bufs=4, name="es")
                                    nc.scalar.activation(e[:, 0:N], ps_s[:, 0:N], AF.Exp)
                                    nc.vector.tensor_tensor(
                                        e[:, 0:P], e[:, 0:P], mdiag[:, r, :], OP.mult
                                    )
                                    nc.tensor.matmul(
                                        oT_ps[hx][:, s_ * P : CH],
                                        vag[:, s_, hh * (HD + 1) : (hh + 1) * (HD + 1)],
                                        e[:, 0:N],
                                        start=(r == 0 and s_ == 0),
                                        stop=(r == RANKS - 1 and s_ == QT - 1),
                                        skip_group_check=True,
                                    )
                        # normalize: divide rows 0:64 by the ones-column sums (row 64)
                        for hx in range(HG):
                            hh = hg * HG + hx
                            pb_ = (hh % 2) * 64
                            od = hh // 2
                            rec_f = wk.tile([1, CH], F32, tag="recf", bufs=2, name="rec_f")
                            nc.vector.reciprocal(rec_f[:], oT_ps[hx][HD : HD + 1, :])
                            rec = wk.tile([1, CH], BF16, tag="rec", bufs=2, name="rec")
                            nc.vector.tensor_copy(out=rec[:], in_=rec_f[:])
                            pb2 = psb.tile([P, CH], F32, tag="sc", bufs=2, name="pb2")
                            nc.tensor.matmul(pb2[0:HD, :], ones_row[:, 0:HD], rec[:], start=True, stop=True)
                            o_raw = wk.tile([HD, CH], BF16, tag="oraw", bufs=2, name="o_raw")
                            nc.vector.tensor_copy(out=o_raw[:], in_=oT_ps[hx][0:HD, :])
                            nc.vector.tensor_tensor(
                                o_all[pb_ : pb_ + 64, od, :], o_raw[:], pb2[0:HD, :], OP.mult
                            )

                    if DBG and l == 0 and _rep == 0:
                        nc.sync.dma_start(dbg_kT[:], kT_w[:])
                        nc.sync.dma_start(dbg_qT[:], qT[:])
                        nc.sync.dma_start(dbg_vw[:], v_w[:].rearrange("p a b c -> p (a b c)"))
                        nc.sync.dma_start(dbg_oall[:], o_all[:])

                    # --- mha^T = wo^T @ o^T, residual add, LN1 ---
                    wof = wk.tile([P, DT, D], BF16, tag="wvf", bufs=2, name="wo_f")
                    nc.sync.dma_start(wof[:], wo_d[l])
                    for dt in range(DT):
                        pm = psb.tile([P, CH], F32, tag="sc", bufs=2, name="ps_wo")
                        for kt in range(DT):
                            nc.tensor.matmul(
                                pm[:], wof[:, kt, dt * P : (dt + 1) * P], o_all[:, kt, :],
                                start=(kt == 0), stop=(kt == DT - 1),
                            )
                        nc.vector.tensor_tensor(h[:, dt, :], h[:, dt, :], pm[:], OP.add)
                    layernorm_d(2 * l)
                    if DBG and l == 0 and _rep == 0:
                        nc.sync.dma_start(dbg_h1[:], h[:])

                    # --- FFN (d-major throughout) ---
                    b1_sb = wk.tile([P, FT], F32, tag="b1s", name="b1_sb")
                    nc.sync.dma_start(b1_sb[:], b1_d[l])
                    b2_sb = wk.tile([1, D], BF16, tag="b2s", name="b2_sb")
                    nc.sync.dma_start(b2_sb[:], b2_d[l : l + 1, :])
                    f1 = wk.tile([P, FT, CH], BF16, tag="f1c", name="f1")
                    for dp in range(FT // 2):
                        w1c = wk.tile([P, DT, 256], BF16, tag="w1s", bufs=2, name="w1c")
                        nc.sync.dma_start(w1c[:], w1_d[l * 12 + dp])
                        for j in range(2):
                            ft = dp * 2 + j
                            pf = psb.tile([P, CH], F32, tag="sc", bufs=2, name="ps_f1")
                            for dt in range(DT):
                                nc.tensor.matmul(
                                    pf[:], w1c[:, dt, j * P : (j + 1) * P], hb[:, dt, :],
                                    start=(dt == 0), stop=(dt == DT - 1),
                                )
                            nc.scalar.activation(
                                f1[:, ft, :], pf[:], AF.Relu, bias=b1_sb[:, ft : ft + 1]
                            )
                    pf2 = [None] * DT
                    for dt in range(DT):
                        pf2[dt] = psb.tile([P, CH], F32, tag="oT", bufs=HG, name="ps_f2")
                        nc.tensor.matmul(
                            pf2[dt][:], b2_sb[:, dt * P : (dt + 1) * P], ones_row[:],
                            start=True, stop=False,
                        )
                    for ft in range(FT):
                        w2c = wk.tile([P, D], BF16, tag="w2s", bufs=3, name="w2c")
                        nc.sync.dma_start(w2c[:], w2_d[l * FT + ft])
                        for dt in range(DT):
                            nc.tensor.matmul(
                                pf2[dt][:], w2c[:, dt * P : (dt + 1) * P], f1[:, ft, :],
                                start=False, stop=(ft == FT - 1),
                            )
                    for dt in range(DT):
                        nc.vector.tensor_tensor(h[:, dt, :], h[:, dt, :], pf2[dt][:], OP.add)
                    layernorm_d(2 * l + 1)
                    if DBG and l == 0 and _rep == 0:
                        nc.sync.dma_start(dbg_h2[:], h[:])

            # ---------- final LN ----------
            with nc.named_scope("final"):
                layernorm_d(NLN - 1)

            # ---------- LM head: own 512 tokens x full (padded) vocab ----------
            _psb_ctx.__exit__(None, None, None)  # release the layer-loop PSUM banks
            if not os.environ.get("TRN_SKIP_LM"):
                with (
                    tc.tile_pool(name="lmp", bufs=1) as lmp,
                    tc.tile_pool(name="pslm", bufs=1, space="PSUM") as pslm,
                    nc.named_scope("lmhead"),
                ):
                    rep_lm = int(os.environ.get("TRN_REP_LM", "1"))
                    for nn_ in range(NVC * rep_lm):
                        n = nn_ % NVC
                        lmw_c = lmp.tile([P, DT, 512], BF16, tag="lmw", bufs=3, name="lmw_c")
                        nc.sync.dma_start(lmw_c[:], lmw_d[n])
                        for qt in range(QT):
                            po = pslm.tile([P, 512], F32, tag="lmo", bufs=4, name="ps_lm")
                            for dt in range(DT):
                                nc.tensor.matmul(
                                    po[:],
                                    hb[:, dt, qt * P : (qt + 1) * P],
                                    lmw_c[:, dt, :],
                                    start=(dt == 0),
                                    stop=(dt == DT - 1),
                                )
                            osb = lmp.tile([P, 512], BF16, tag="osb", bufs=4, name="o_sb")
                            nc.vector.tensor_copy(out=osb[:], in_=po[:])
                            nc.sync.dma_start(
                                logits[(n * QT + qt) * P : (n * QT + qt + 1) * P, :], osb[:]
                            )

    return _finish(nc)


def _finish(nc):
    nc.compile()
    return nc


def _pe_table():
    pos = np.arange(S, dtype=np.float32)[:, None]
    div = np.exp(np.arange(0, D, 2, dtype=np.float32) * (-math.log(10000.0) / D))
    pe = np.zeros((S, D), dtype=np.float32)
    pe[:, 0::2] = np.sin(pos * div)
    pe[:, 1::2] = np.cos(pos * div)
    return pe


def prep_in_maps(inputs):
    """Host-side preprocessing: embedding gather + PE, casts, and repacking
    of every tensor into the kernel's SBUF-native layouts."""
    x = np.asarray(inputs["x"])
    bf = ml_dtypes.bfloat16

    def pack_po(w):
        # [rows, cols] with rows = T*128 -> [128, T, cols] bf16
        w = np.asarray(w, dtype=np.float32)
        t = w.shape[0] // P
        return np.ascontiguousarray(w.reshape(t, P, w.shape[1]).transpose(1, 0, 2).astype(bf))

    def pack_qkv(w):
        w = np.asarray(w, dtype=np.float32).transpose(0, 2, 1, 3).reshape(NL, D, D)
        return np.stack([pack_po(w[i]) for i in range(NL)])

    wq = pack_qkv(inputs["wq"])
    wk_ = pack_qkv(inputs["wk"])
    wv = pack_qkv(inputs["wv"])
    wo = np.stack([pack_po(np.asarray(inputs["wo"], dtype=np.float32)[i]) for i in range(NL)])

    w1f = np.asarray(inputs["w1"], dtype=np.float32)  # [NL, D, DF]
    w1p = w1f.reshape(NL, DT, P, 12, 256).transpose(0, 3, 2, 1, 4)
    w1p = np.ascontiguousarray(w1p.reshape(NL * 12, P, DT, 256).astype(bf))
    w2f = np.asarray(inputs["w2"], dtype=np.float32)  # [NL, DF, D]
    w2p = np.ascontiguousarray(w2f.reshape(NL * FT, P, D).astype(bf))
    b1p = np.ascontiguousarray(
        np.asarray(inputs["b1"], dtype=np.float32).reshape(NL, FT, P).transpose(0, 2, 1)
    )

    lmw = np.zeros((D, VP), dtype=np.float32)
    lmw[:, :V] = np.asarray(inputs["lm_w"], dtype=np.float32)
    lmw_pack = np.ascontiguousarray(lmw.reshape(DT, P, NVC, 512).transpose(2, 1, 0, 3).astype(bf))

    # layernorm params, d-major per-partition: [128, (ln,g/b,dt)]
    lnp = np.zeros((P, NLN, 2, DT), dtype=np.float32)
    for l in range(NL):
        for i, (g, b) in enumerate([("ln1_g", "ln1_b"), ("ln2_g", "ln2_b")]):
            lnp[:, 2 * l + i, 0, :] = np.asarray(inputs[g], np.float32)[l].reshape(DT, P).T
            lnp[:, 2 * l + i, 1, :] = np.asarray(inputs[b], np.float32)[l].reshape(DT, P).T
    lnp[:, NLN - 1, 0, :] = np.asarray(inputs["lnf_g"], np.float32).reshape(DT, P).T
    lnp[:, NLN - 1, 1, :] = np.asarray(inputs["lnf_b"], np.float32).reshape(DT, P).T
    lnp = np.ascontiguousarray(lnp.reshape(P, NLN * 2 * DT))

    common = {
        "wq": wq, "wk": wk_, "wv": wv, "wo": wo,
        "w1": w1p, "w2": w2p, "b1": b1p,
        "b2": np.ascontiguousarray(np.asarray(inputs["b2"], np.float32).astype(bf)),
        "lnp": lnp,
        "lm_w": lmw_pack,
    }

    pe = _pe_table()
    tok_emb = np.asarray(inputs["tok_emb"], dtype=np.float32)
    triu_s = np.triu(np.ones((P, P), np.float32), 1)  # es rows = k, cols = q: k < q
    eye = np.eye(P, dtype=np.float32)

    in_maps = []
    for c in range(NC):
        b, j = c // RANKS, c % RANKS
        pos = 4 * np.arange(CH) + j  # own (strided) token positions
        h0 = tok_emb[x[b, pos]] + pe[pos]  # [512, 768]
        h0 = np.ascontiguousarray(h0.T.reshape(DT, P, CH).transpose(1, 0, 2))
        md = np.stack([triu_s + (r <= j) * eye for r in range(RANKS)])  # [4,128,128]
        md = np.ascontiguousarray(md.transpose(1, 0, 2).astype(bf))
        in_maps.append({**common, "h0": h0, "mdiag": md})
    return in_maps


def postprocess(res, inputs):
    """[NVC*QT*128, 512] bf16 per core -> [B, S, V] f32 (+ lm_b on host)."""
    out = np.empty((B, S, V), dtype=np.float32)
    for c in range(NC):
        b, j = c // RANKS, c % RANKS
        a = np.asarray(res[c]["logits"], dtype=np.float32)
        a = a.reshape(NVC, QT, P, 512).transpose(1, 2, 0, 3).reshape(CH, VP)
        out[b, 4 * np.arange(CH) + j] = a[:, :V]
    lm_b = np.asarray(inputs["lm_b"], dtype=np.float32)
    if lm_b.any():
        out += lm_b
    return out


def kernel(**inputs):
    if "nc" not in _CACHE:
        _CACHE["nc"] = build()
    nc = _CACHE["nc"]

    in_maps = prep_in_maps(inputs)

    trace = bool(os.environ.get("TRN_KERNEL_TRACE"))
    res = run_bass_kernel_spmd(nc, in_maps, core_ids=list(range(NC)), trace=trace)
    _CACHE["last_result"] = res
    _CACHE["last_in_maps"] = in_maps
    return postprocess(res.results, inputs)


if __name__ == "__main__":
    import time

    t0 = time.time()
    nc = build()
    print(f"build ok: {time.time() - t0:.1f}s")
